# revision 20
# baseline (speedup 1.0000x reference)
"""Bass/Trainium2 kernel for nn_BiGRIL (gnn_message_passing).

Algebraic structure (h == 0, C == 1):
  x1   = where(mask, x, b_fs)
  z    = W0*x1 + W1*m + b_in            (rank-2 in channels)
  zg   = A^T z  ->  W0*xg + W1*mg + b_in*cg   with xg = A^T x1, mg = A^T m,
                                              cg = A^T 1
  v    = M1 z + M2 zg + b_fold          (K=5 matmul + bias via ACT)
  o    = PReLU(v)                       (ScalarE Prelu, bias_v folded in)
  w    = PB^T o                         (pass B)
  rr   = max(w + bias_f + k, k)         (VectorE TS; k-fold absorbs b_o2)
  out  = W_o2 . rr                      (pass C)

v3 "paired-lane" schedule: chunk pair (g, g+64) packs two 512-position
chunks into the 128 psum partitions (lane0 -> 0:64, lane1 -> 64:128):
one K=128 matmul per pair per pass, one ScalarE Prelu per pair, one
VectorE TS per pair.  Startup DMAs are consolidated into a few large
multi-dim-AP transfers; 40 warmup matmuls hold the PE HAM clock at 8/8.

Sharding: data-parallel over batch (B=8 -> 8 cores), no collectives.
"""

import numpy as np
import sys

sys.path.insert(0, "/opt/trn_rl_repo")

B, C, N, T = 8, 1, 1024, 64
H = 64
NT = N * T          # 65536 per-core output elements
CHUNK = 512
NPAIR = 64          # pairs (g, g+64); chunk c=g is lane0, c=g+64 lane1
HALF = NT // 2      # 32768: lane1 position offset
BLK = 4096          # ma tile columns (64 nodes x 64 steps)
NBLK = 8            # blocks per lane

_CACHE = {}


def _fold_weights(W_fs, b_fs, W_in, b_in, W_gc, b_gc, W_lo, b_lo, prelu_a,
                  W_ro, b_ro, W_o1, b_o1, W_o2, b_o2, adj):
    """Host-side weight folding in float64 for accuracy."""
    f8 = np.float64
    W_in, b_in = W_in.astype(f8), b_in.astype(f8)
    W_gc, b_gc = W_gc.astype(f8), b_gc.astype(f8)
    W_lo, b_lo = W_lo.astype(f8), b_lo.astype(f8)
    W_ro, b_ro = W_ro.astype(f8), b_ro.astype(f8)
    W_o1, b_o1 = W_o1.astype(f8), b_o1.astype(f8)
    W_o2, b_o2 = W_o2.astype(f8), b_o2.astype(f8)

    W0 = W_in[:, 0]           # x1 channel  [64]
    W1 = W_in[:, 1]           # mask channel [64]
    Wlo1 = W_lo[:, :H]
    M1 = Wlo1 @ W_gc[:, :H]
    M2 = Wlo1 @ W_gc[:, H:]
    b_fold = Wlo1 @ b_gc + b_lo

    PA = np.stack([
        M1 @ W0,
        M1 @ W1,
        M2 @ W0,
        M2 @ W1,
        M2 @ b_in,
    ])                                     # [5, 64]  lhsT for pass A
    bias_v = M1 @ b_in + b_fold            # [64] -> Prelu bias

    w_ro1 = W_ro[0, :H]                    # [64]
    PB = np.outer(w_ro1, W_o1[:, 0])       # [64(h), 64(f)] lhsT for pass B
    bias_f = W_o1[:, 0] * b_ro[0] + b_o1   # [64]

    den = float(np.sum(W_o2[0] ** 2))
    assert abs(den) > 1e-12
    k = float(b_o2[0]) * W_o2[0] / den     # fold b_o2: W_o2.(rr + k) adds b_o2

    cg = adj.astype(f8).sum(axis=0)        # [N] column sums of adj
    cgrep = np.repeat(cg, T)               # [(n,t)] layout n*T + t

    a = float(prelu_a)
    assert 0.0 < a < 1.0

    # pass A stationary: lane0 streams at rows 0:5 -> out 0:64,
    #                    lane1 streams at rows 32:37 -> out 64:128
    paA = np.zeros((128, 128))
    paA[0:5, 0:64] = PA
    paA[32:37, 64:128] = PA
    # pass B stationary: blockdiag(PB, PB) over packed prelu [128, 512]
    pbB = np.zeros((128, 128))
    pbB[0:64, 0:64] = PB
    pbB[64:128, 64:128] = PB
    # pass C stationary: col 0 <- lane0 W_o2, col 32 <- lane1 W_o2
    pcC = np.zeros((128, 64))
    pcC[0:64, 0] = W_o2[0]
    pcC[64:128, 32] = W_o2[0]

    h16 = np.float16
    fp = np.float32
    # consolidated h16 consts [128, 320]: pa | pb | pc
    ch = np.zeros((128, 320))
    ch[:, 0:128] = paA
    ch[:, 128:256] = pbB
    ch[:, 256:320] = pcC
    # consolidated f32 consts [128, 4]: bk | kk | bfs | bias_v (2 lanes)
    cf = np.zeros((128, 4))
    cf[0:64, 0] = bias_f + k
    cf[64:128, 0] = bias_f + k
    cf[0:64, 1] = k
    cf[64:128, 1] = k
    cf[:, 2] = b_fs[0]
    cf[0:64, 3] = bias_v
    cf[64:128, 3] = bias_v

    return dict(
        consts_h=ch.astype(h16),
        consts_f=cf.astype(fp),
        cgrep=cgrep.astype(h16),
        prelu_a=a,
    )


def _build_program(prelu_a):
    import concourse.bass as bass
    import concourse.bacc as bacc
    import concourse.mybir as mybir
    import concourse.tile as tile

    dt = mybir.dt
    f32 = dt.float32
    h16 = dt.float16
    AF = mybir.ActivationFunctionType
    ALU = mybir.AluOpType

    nc = bacc.Bacc("TRN2", target_bir_lowering=False, debug=False,
                   num_devices=B)

    xb = nc.dram_tensor("xb", [N, T], h16, kind="ExternalInput")
    mb = nc.dram_tensor("mb", [N, T], h16, kind="ExternalInput")
    adj = nc.dram_tensor("adj", [N, N], h16, kind="ExternalInput")
    cgrep = nc.dram_tensor("cgrep", [NT], h16, kind="ExternalInput")
    ch_d = nc.dram_tensor("consts_h", [128, 320], h16, kind="ExternalInput")
    cf_d = nc.dram_tensor("consts_f", [128, 4], f32, kind="ExternalInput")
    out_d = nc.dram_tensor("out", [NT], f32, kind="ExternalOutput")

    MOVA_BUFS = 4
    from contextlib import ExitStack
    with tile.TileContext(nc) as tc, ExitStack() as ctx:
        const = ctx.enter_context(tc.tile_pool(name="const", bufs=1))
        adjp = ctx.enter_context(tc.tile_pool(name="adjp", bufs=1))
        gmovp = ctx.enter_context(tc.tile_pool(name="gmovp", bufs=1))
        movap = ctx.enter_context(tc.tile_pool(name="movap", bufs=1))
        p2p = ctx.enter_context(tc.tile_pool(name="p2p", bufs=4))
        rrp = ctx.enter_context(tc.tile_pool(name="rrp", bufs=4))
        posbp = ctx.enter_context(tc.tile_pool(name="posbp", bufs=3))
        vps = ctx.enter_context(tc.tile_pool(name="vps", bufs=2, space="PSUM"))
        wps = ctx.enter_context(tc.tile_pool(name="wps", bufs=2, space="PSUM"))
        pop = ctx.enter_context(tc.tile_pool(name="pop", bufs=2, space="PSUM"))
        gps = ctx.enter_context(tc.tile_pool(name="gps", bufs=2, space="PSUM"))

        # ---- consolidated constant loads (2 DMAs) ---------------------
        ch_t = const.tile([128, 320], h16)
        cf_t = const.tile([128, 4], f32)
        nc.sync.dma_start(out=ch_t[:], in_=ch_d[:])
        nc.sync.dma_start(out=cf_t[:], in_=cf_d[:])
        pa_t = ch_t[:, 0:128]
        pb_t = ch_t[:, 128:256]
        pc_t = ch_t[:, 256:320]
        bk_t = cf_t[:, 0:1]
        kk_t = cf_t[:, 1:2]
        bfs_t = cf_t[:, 2:3]
        bv_t = cf_t[:, 3:4]

        # ---- x/m: one DMA each into the combined gmov tile ------------
        # gmov[p, nt*128 + 0:64]  = x[node nt*128+p, t]
        # gmov[p, nt*128 + 64:128] = m[node nt*128+p, t]
        gmov = gmovp.tile([128, 1024], h16, tag="gmov", name="gmov")
        g3 = gmov[:, :].rearrange("p (nt ts) -> p nt ts", ts=128)
        nc.sync.dma_start(out=g3[:, :, 0:64],
                          in_=xb[:, :].rearrange("(nt p) t -> p nt t", p=128))
        nc.sync.dma_start(out=g3[:, :, 64:128],
                          in_=mb[:, :].rearrange("(nt p) t -> p nt t", p=128))
        # x1 = (x - bfs)*m + bfs  on the x columns (strided, one op each)
        gx_cols = g3[:, :, 0:64]
        gm_cols = g3[:, :, 64:128]
        nc.vector.scalar_tensor_tensor(
            out=gx_cols, in0=gx_cols, scalar=bfs_t,
            in1=gm_cols, op0=ALU.subtract, op1=ALU.mult)
        nc.vector.tensor_scalar_add(gx_cols, gx_cols, bfs_t)

        # ---- adj: 3 large DMAs into one combined tile -----------------
        # adjt[p, nt*1024 + c] = adj[nt*128 + p, c]
        adjt = adjp.tile([128, 8192], h16, tag="adjt", name="adjt")
        at3 = adjt[:, :].rearrange("p (nt c) -> p nt c", c=1024)
        nc.sync.dma_start(
            out=at3[:, 0:3, :],
            in_=adj[0:384, :].rearrange("(nt p) c -> p nt c", p=128))
        nc.gpsimd.dma_start(
            out=at3[:, 3:6, :],
            in_=adj[384:768, :].rearrange("(nt p) c -> p nt c", p=128))
        nc.scalar.dma_start(
            out=at3[:, 6:8, :],
            in_=adj[768:1024, :].rearrange("(nt p) c -> p nt c", p=128))

        # ---- HAM warmup: keep PE streaming so the clock hits 8/8 ------
        warm_ps = gps.tile([128, 512], f32, tag="psg", name="warm")
        for wi in range(40):
            nc.tensor.matmul(warm_ps[:, 0:128], pa_t, pb_t,
                             start=True, stop=True, skip_group_check=True)

        # ---- ma tiles: 4 persistent [128, 4096] fp16 ------------------
        # rows 0:5   = lane0 streams (x1, m, xg, mg, cg)
        # rows 32:37 = lane1 streams (nodes 512:1024)
        # all other rows zero (K=128 matmuls; stationary rows are zero)
        ma4 = []
        for i in range(MOVA_BUFS):
            mai = movap.tile([128, BLK], h16, tag=f"mova{i}", name=f"mova{i}")
            eng = nc.vector if i % 2 == 0 else nc.gpsimd
            eng.memset(mai[:, :], 0.0)
            ma4.append(mai)

        out3 = out_d[:].rearrange("(l g c) -> g l c", l=2, c=CHUNK)
        gx = [None] * 8
        ma_t = [None] * NBLK
        vps_t = {}
        wps_t = {}
        p2_t = {}
        rr_t = {}
        po_ps = {}

        def emit_g(mt):
            psg = gps.tile([128, 512], f32, tag="psg", name=f"psg{mt}")
            for nt in range(8):
                nc.tensor.matmul(
                    psg[:, 0:128],
                    adjt[:, nt * 1024 + mt * 128:nt * 1024 + (mt + 1) * 128],
                    gmov[:, nt * 128:(nt + 1) * 128],
                    start=(nt == 0), stop=(nt == 7))
            g = gmovp.tile([128, 128], h16, tag=f"gxm{mt}", name=f"gxm{mt}")
            nc.scalar.activation(g[:], psg[:, 0:128], AF.Copy,
                                 bias=0.0, scale=1.0)
            gx[mt] = g

        cg2 = cgrep[:].rearrange("(l q) -> l q", l=2)

        def emit_ma(blk):
            # lane0: nodes [blk*64, blk*64+64) -> gmov/gx block blk//2,
            #        half p0 = (blk%2)*64;  lane1: nodes +512
            # combined 2-row gathers: [64 nodes, 128(2x64 t)] -> 2 ma rows
            mt0, mt1 = blk // 2, 4 + blk // 2
            p0 = (blk % 2) * 64
            ma = ma4[blk % MOVA_BUFS]
            c0, c1 = mt0 * 128, mt1 * 128
            nc.sync.dma_start(out=ma[0:1, :], in_=gmov[p0:p0 + 64, c0:c0 + 64])
            nc.sync.dma_start(out=ma[1:2, :],
                              in_=gmov[p0:p0 + 64, c0 + 64:c0 + 128])
            nc.gpsimd.dma_start(out=ma[2:3, :], in_=gx[mt0][p0:p0 + 64, 0:64])
            nc.sync.dma_start(out=ma[3:4, :], in_=gx[mt0][p0:p0 + 64, 64:128])
            nc.gpsimd.dma_start(out=ma[4:5, :],
                                in_=cg2[0:1, blk * BLK:(blk + 1) * BLK])
            nc.gpsimd.dma_start(out=ma[32:33, :],
                                in_=gmov[p0:p0 + 64, c1:c1 + 64])
            nc.sync.dma_start(out=ma[33:34, :],
                              in_=gmov[p0:p0 + 64, c1 + 64:c1 + 128])
            nc.gpsimd.dma_start(out=ma[34:35, :], in_=gx[mt1][p0:p0 + 64, 0:64])
            nc.sync.dma_start(out=ma[35:36, :],
                              in_=gx[mt1][p0:p0 + 64, 64:128])
            nc.gpsimd.dma_start(out=ma[36:37, :],
                                in_=cg2[1:2, blk * BLK:(blk + 1) * BLK])
            ma_t[blk] = ma

        def st_a(g):
            blk, j = g // 8, g % 8
            if g == 0:
                emit_g(0)
                emit_g(4)
                emit_ma(0)
                # second warmup burst: keep the PE busy (HAM warm) while
                # the block-0 gathers land
                warm2 = gps.tile([128, 512], f32, tag="psg", name="warm2")
                for wi in range(25):
                    nc.tensor.matmul(warm2[:, 0:128], pa_t, pb_t,
                                     start=True, stop=True,
                                     skip_group_check=True)
            if j == 0 and blk + 1 < NBLK:
                if (blk + 1) % 2 == 0:
                    emit_g((blk + 1) // 2)
                    emit_g(4 + (blk + 1) // 2)
                emit_ma(blk + 1)
            c0 = j * CHUNK
            ps_v = vps.tile([128, 512], f32, tag="v", name=f"v{g}")
            nc.tensor.matmul(ps_v[:], pa_t, ma_t[blk][:, c0:c0 + CHUNK],
                             start=True, stop=True)
            p2 = p2p.tile([128, CHUNK], h16, tag="p2", name=f"p2{g}")
            nc.scalar.activation(p2[:], ps_v[:], AF.Prelu,
                                 bias=bv_t, scale=1.0, alpha=prelu_a)
            vps_t[g] = ps_v
            p2_t[g] = p2

        def st_b(g):
            del vps_t[g]
            ps_w = wps.tile([128, 512], f32, tag="w", name=f"w{g}")
            nc.tensor.matmul(ps_w[:], pb_t, p2_t[g][:],
                             start=True, stop=True)
            del p2_t[g]
            rr = rrp.tile([128, CHUNK], h16, tag="rr", name=f"rr{g}")
            nc.vector.tensor_scalar(
                out=rr[:, :], in0=ps_w[:, :],
                scalar1=bk_t, scalar2=kk_t,
                op0=ALU.add, op1=ALU.max)
            wps_t[g] = ps_w
            rr_t[g] = rr

        def st_c(g):
            del wps_t[g]
            q = g % 2
            if q == 0:
                po_ps[g // 2] = pop.tile([128, 512], f32, tag="po",
                                         name=f"po{g // 2}")
            # pair g outputs: row 64q = lane0 (chunk g), row 64q+32 = lane1
            nc.tensor.matmul(po_ps[g // 2][64 * q:64 * q + 64, :], pc_t,
                             rr_t[g][:, :], start=True, stop=True,
                             tile_position=(0, 64 * q))
            del rr_t[g]
            if q == 1:
                bi = g // 2
                po_sb = posbp.tile([97, 512], f32, tag="po_sb",
                                   name=f"po_sb{bi}")
                if bi % 2 == 0:
                    nc.scalar.activation(po_sb[0:97, :], po_ps[bi][0:97, :],
                                         AF.Identity, bias=0.0, scale=1.0)
                else:
                    nc.vector.tensor_copy(po_sb[0:97, :], po_ps[bi][0:97, :])
                del po_ps[bi]
                # rows (0,32,64,96) = (pair g-1 lane0, g-1 lane1, g l0, g l1)
                # -> one DMA: dram dims (pair: 512, lane: HALF, elem: 1)
                eng = nc.sync if bi % 2 == 0 else nc.gpsimd
                eng.dma_start(out=out3[g - 1:g + 1], in_=po_sb[0:97:32, :])

        for p in range(NPAIR + 4):
            if p < NPAIR:
                st_a(p)
            if 2 <= p < NPAIR + 2:
                st_b(p - 2)
            if p >= 4:
                st_c(p - 4)

    nc.compile()
    return nc


def _get_program(prelu_a):
    key = ("prog", float(prelu_a))
    if key not in _CACHE:
        _CACHE[key] = _build_program(prelu_a)
    return _CACHE[key]


def make_in_maps(x, mask, W_fs, b_fs, W_in, b_in, adj, W_gc, b_gc, W_lo, b_lo,
                 prelu_a, W_ro, b_ro, W_o1, b_o1, W_o2, b_o2):
    x = np.asarray(x, np.float32)
    mask_f = np.asarray(mask, np.float16)
    adj = np.asarray(adj, np.float32)

    folded = _fold_weights(np.asarray(W_fs), np.asarray(b_fs),
                           np.asarray(W_in), np.asarray(b_in),
                           np.asarray(W_gc), np.asarray(b_gc),
                           np.asarray(W_lo), np.asarray(b_lo),
                           float(prelu_a),
                           np.asarray(W_ro), np.asarray(b_ro),
                           np.asarray(W_o1), np.asarray(b_o1),
                           np.asarray(W_o2), np.asarray(b_o2), adj)

    shared = dict(adj=adj.astype(np.float16), cgrep=folded["cgrep"],
                  consts_h=folded["consts_h"], consts_f=folded["consts_f"])
    in_maps = []
    for b in range(B):
        m = dict(shared)
        m["xb"] = np.ascontiguousarray(x[b, 0]).astype(np.float16)
        m["mb"] = np.ascontiguousarray(mask_f[b, 0])
        in_maps.append(m)
    return in_maps, folded["prelu_a"]


def kernel(x, mask, W_fs, b_fs, W_in, b_in, adj, W_gc, b_gc, W_lo, b_lo,
           prelu_a, W_ro, b_ro, W_o1, b_o1, W_o2, b_o2):
    in_maps, a = make_in_maps(x, mask, W_fs, b_fs, W_in, b_in, adj, W_gc,
                              b_gc, W_lo, b_lo, prelu_a, W_ro, b_ro, W_o1,
                              b_o1, W_o2, b_o2)
    nc = _get_program(a)

    from concourse.bass_utils import run_bass_kernel_spmd
    res = run_bass_kernel_spmd(nc, in_maps, list(range(B)))

    out = np.empty((B, C, N, T), np.float32)
    for b in range(B):
        out[b, 0] = np.asarray(res.results[b]["out"]).reshape(N, T)
    return out
